# revision 6
# baseline (speedup 1.0000x reference)
"""Trainium2 Bass kernel for VITS-style relative-position MultiHeadAttention.

Problem: B=4, T=1024, C=512, H=8 heads, d=64, window=4 relative attention
(rel embeddings shared across heads). Sharded over 8 NeuronCores as
(batch x head-group): core = 2*b + hg, each core handles batch b and 4 heads.

Layout strategy per core:
  - scores computed TRANSPOSED: scoresT[s, t] = k[s] . q_scaled[t]
    (s-tile on partitions) so that PV contraction uses v as the matmul
    stationary operand and E^T as the moving operand.
  - softmax without max-subtraction (scores ~ N(0,1), fp32 exp is safe);
    denominator from a ones-column appended to the v stationary.
  - the 9-diagonal relative-K band is added pre-exp: RL9T[j,t] = emb_k[j].q_s[t]
    matmul -> 9 shifted row DMAs -> PE transpose -> per-partition
    gpsimd.local_scatter builds dense [128,136] band windows -> DVE add.
  - the band of the post-exp attention (needed for the rel-V term) is
    extracted via a skewed DRAM bounce buffer G where every DMA is
    contiguous-run (window writes = 544B runs, band reads = 36B runs),
    then the inverse skew (PE transpose + 9 shifted row DMAs) produces
    AbandT[9, t], contracted against emb_v directly into the PV PSUM.
  - normalization (1/rowsum) is applied per head after the output
    projection: out = sum_h recip_h[t] * (outT_h.T @ Wo_h), recip
    transposed to column layout via PE transposes.
"""

import numpy as np

import concourse.bass as bass
import concourse.bacc as bacc
import concourse.mybir as mybir
import concourse.tile as tile
from concourse.bass_utils import run_bass_kernel_spmd
from concourse.masks import make_identity

f32 = mybir.dt.float32
bf16 = mybir.dt.bfloat16
i16 = mybir.dt.int16

T = 1024          # sequence length (t_t == t_s)
CIN = 512         # input channels
CH = 256          # channels per core (head group)
NHEADS = 4        # heads per core
D = 64            # head dim
NB = 9            # band width (2*window+1)
NT = T // 128     # 8 tiles of 128
GPITCH = 137      # G buffer row pitch (136 + 1)
GSZ = T * GPITCH + 256

Exp = mybir.ActivationFunctionType.Exp
Identity = mybir.ActivationFunctionType.Identity
Copy = mybir.ActivationFunctionType.Copy
AluAdd = mybir.AluOpType.add
AluMult = mybir.AluOpType.mult


def build_program():
    nc = bacc.Bacc()

    # ---- external I/O (per-core shapes) ----
    xT = nc.declare_dram_parameter("xT", [CIN, T], f32, isOutput=False)
    cT = nc.declare_dram_parameter("cT", [CIN, T], f32, isOutput=False)
    wq = nc.declare_dram_parameter("wq", [CIN, CH], f32, isOutput=False)
    wk = nc.declare_dram_parameter("wk", [CIN, CH], f32, isOutput=False)
    wv = nc.declare_dram_parameter("wv", [CIN, CH], f32, isOutput=False)
    wo = nc.declare_dram_parameter("wo", [CH, CIN], f32, isOutput=False)
    bq2 = nc.declare_dram_parameter("bq2", [128, 2], f32, isOutput=False)
    bk2 = nc.declare_dram_parameter("bk2", [128, 2], f32, isOutput=False)
    bv1 = nc.declare_dram_parameter("bv1", [1, CH], f32, isOutput=False)
    ekT = nc.declare_dram_parameter("ekT", [D, NB], f32, isOutput=False)
    ev = nc.declare_dram_parameter("ev", [NB, D], f32, isOutput=False)
    sidx = nc.declare_dram_parameter("sidx", [128, 10], i16, isOutput=False)
    out_p = nc.declare_dram_parameter("out_p", [T, CIN], f32, isOutput=True)

    with tile.TileContext(nc) as tc:
        with (
            tc.tile_pool(name="const", bufs=1) as cpool,
            tc.tile_pool(name="win", bufs=1) as wpool,
            tc.tile_pool(name="xin", bufs=1) as xpool,
            tc.tile_pool(name="qk", bufs=1) as qkpool,
            tc.tile_pool(name="vaug", bufs=1) as vpool,
            tc.tile_pool(name="band", bufs=1) as bpool,
            tc.tile_pool(name="et", bufs=10) as etpool,
            tc.tile_pool(name="outp", bufs=1) as opool,
            tc.tile_pool(name="dram", bufs=1, space="DRAM") as dpool,
        ):
            # ---------- constants ----------
            ident = cpool.tile([128, 128], f32)
            make_identity(nc, ident[:])
            ones1 = cpool.tile([1, 128], f32)
            nc.gpsimd.memset(ones1[:], 1.0)
            sidx_sb = cpool.tile([128, 10], i16)
            nc.sync.dma_start(sidx_sb[:], sidx[:])
            ekT_sb = cpool.tile([128, NB], f32)
            nc.sync.dma_start(ekT_sb[0:D, :], ekT[:])
            nc.sync.dma_start(ekT_sb[D:2 * D, :], ekT[:])
            ev_sb = cpool.tile([NB, D + 1], f32)
            nc.gpsimd.memset(ev_sb[:], 0.0)
            nc.sync.dma_start(ev_sb[:, 0:D], ev[:])
            bq_sb = cpool.tile([128, 2], f32)
            nc.sync.dma_start(bq_sb[:], bq2[:])
            bk_sb = cpool.tile([128, 2], f32)
            nc.sync.dma_start(bk_sb[:], bk2[:])
            bv_sb = cpool.tile([1, CH], f32)
            nc.sync.dma_start(bv_sb[:], bv1[:])

            # ---------- load weights + inputs ----------
            wq_sb = []
            wk_sb = []
            wv_sb = []
            xT_sb = []
            cT_sb = []
            for kt in range(4):
                t_ = wpool.tile([128, CH], f32, tag=f"wq{kt}")
                nc.sync.dma_start(t_[:], wq[kt * 128:(kt + 1) * 128, :])
                wq_sb.append(t_)
                t_ = wpool.tile([128, CH], f32, tag=f"wk{kt}")
                nc.sync.dma_start(t_[:], wk[kt * 128:(kt + 1) * 128, :])
                wk_sb.append(t_)
                t_ = wpool.tile([128, CH], f32, tag=f"wv{kt}")
                nc.sync.dma_start(t_[:], wv[kt * 128:(kt + 1) * 128, :])
                wv_sb.append(t_)
                t_ = xpool.tile([128, T], f32, tag=f"xT{kt}")
                nc.sync.dma_start(t_[:], xT[kt * 128:(kt + 1) * 128, :])
                xT_sb.append(t_)
                t_ = xpool.tile([128, T], f32, tag=f"cT{kt}")
                nc.sync.dma_start(t_[:], cT[kt * 128:(kt + 1) * 128, :])
                cT_sb.append(t_)
            wo_sb = []
            for ct in range(2):
                t_ = wpool.tile([128, CIN], f32, tag=f"wo{ct}")
                nc.sync.dma_start(t_[:], wo[ct * 128:(ct + 1) * 128, :])
                wo_sb.append(t_)

            # ---------- phase A: QKV projections ----------
            qsT_sb = [qkpool.tile([128, T], f32, tag=f"qsT{ct}", name=f"qsT{ct}") for ct in range(2)]
            kT_sb = [qkpool.tile([128, T], f32, tag=f"kT{ct}", name=f"kT{ct}") for ct in range(2)]
            with tc.tile_pool(name="psA", bufs=4, space="PSUM") as psA:
                for ct in range(2):
                    for nh in range(2):
                        ps = psA.tile([128, 512], f32, tag="qk")
                        for kt in range(4):
                            nc.tensor.matmul(
                                ps[:],
                                wq_sb[kt][:, ct * 128:(ct + 1) * 128],
                                xT_sb[kt][:, nh * 512:(nh + 1) * 512],
                                start=(kt == 0), stop=(kt == 3),
                            )
                        # q_scaled = (x@Wq)*0.125 + bq*0.125  (bq2 pre-scaled)
                        nc.scalar.activation(
                            qsT_sb[ct][:, nh * 512:(nh + 1) * 512], ps[:],
                            Identity, bias=bq_sb[:, ct:ct + 1], scale=0.125,
                        )
                        ps = psA.tile([128, 512], f32, tag="qk")
                        for kt in range(4):
                            nc.tensor.matmul(
                                ps[:],
                                wk_sb[kt][:, ct * 128:(ct + 1) * 128],
                                cT_sb[kt][:, nh * 512:(nh + 1) * 512],
                                start=(kt == 0), stop=(kt == 3),
                            )
                        nc.scalar.activation(
                            kT_sb[ct][:, nh * 512:(nh + 1) * 512], ps[:],
                            Identity, bias=bk_sb[:, ct:ct + 1], scale=1.0,
                        )
                # v natural [s, ch] + ones column per head -> [128, 4*65]
                vaug_sb = []
                for st in range(NT):
                    va = vpool.tile([128, NHEADS * (D + 1)], f32, tag=f"va{st}")
                    nc.gpsimd.memset(va[:], 1.0)
                    ps = psA.tile([128, CH], f32, tag="v")
                    for kt in range(4):
                        nc.tensor.matmul(
                            ps[:],
                            cT_sb[kt][:, st * 128:(st + 1) * 128],
                            wv_sb[kt][:],
                            start=(kt == 0), stop=False,
                        )
                    nc.tensor.matmul(ps[:], ones1[:], bv_sb[:], start=False, stop=True)
                    nc.vector.tensor_copy(
                        va[:].rearrange("p (h c) -> p h c", h=NHEADS)[:, :, 0:D],
                        ps[:].rearrange("p (h c) -> p h c", h=NHEADS),
                    )
                    vaug_sb.append(va)

            # ---------- phase B: rel-K band prep ----------
            # RL9T[j, t] = emb_k[j] . q_scaled[t]  per head -> skewed S tiles
            s4t_cat = bpool.tile([64, T], f32, tag="s4t")
            nc.gpsimd.memset(s4t_cat[:], 0.0)
            with tc.tile_pool(name="psB", bufs=2, space="PSUM") as psB:
                for h in range(NHEADS):
                    ct, r0 = h // 2, (h % 2) * 64
                    rl = psB.tile([NB, T], f32, tag="rl9")
                    for nh in range(2):
                        nc.tensor.matmul(
                            rl[:, nh * 512:(nh + 1) * 512],
                            ekT_sb[r0:r0 + D, :],
                            qsT_sb[ct][r0:r0 + 64, nh * 512:(nh + 1) * 512],
                            start=True, stop=True,
                        )
                    rlp = bpool.tile([NB, T + 8], f32, tag=f"rl9p{h}")
                    nc.gpsimd.memset(rlp[:, 0:4], 0.0)
                    nc.gpsimd.memset(rlp[:, T + 4:T + 8], 0.0)
                    nc.vector.tensor_copy(rlp[:, 4:T + 4], rl[:])
                    # S^T rows: S^T[h*16+m, p'] = RL9T[8-m, p'+m-4] (padded idx p'+m)
                    for m in range(NB):
                        nc.sync.dma_start(
                            s4t_cat[h * 16 + m:h * 16 + m + 1, :],
                            rlp[8 - m:8 - m + 1, m:m + T],
                        )
                # transpose to S [128(s), .] and cast to bf16 scatter sources
                sbf = []
                for st in range(NT):
                    ps = psB.tile([128, 64], f32, tag="s4tp")
                    nc.tensor.transpose(
                        ps[:], s4t_cat[:, st * 128:(st + 1) * 128], ident[0:64, 0:64]
                    )
                    sb = bpool.tile([128, NHEADS * 10], bf16, tag=f"sbf{st}")
                    nc.gpsimd.memset(sb[:], 0.0)
                    nc.vector.tensor_copy(
                        sb[:].rearrange("p (h c) -> p h c", h=NHEADS)[:, :, 0:NB],
                        ps[:].rearrange("p (h c) -> p h c", h=NHEADS)[:, :, 0:NB],
                    )
                    sbf.append(sb)

            # G bounce buffers (skewed band storage), one per head
            g_dram = [dpool.tile([1, GSZ], f32, tag=f"g{h}", name=f"g{h}") for h in range(NHEADS)]
            zeros_sb = cpool.tile([1, 40], f32)
            nc.gpsimd.memset(zeros_sb[:], 0.0)
            for h in range(NHEADS):
                gt = g_dram[h]
                # zero the band cells of rows 0..3 and 1020..1023 (t out of range)
                nc.sync.dma_start(
                    bass.AP(gt[:].tensor, gt[:].offset, [[GPITCH, 4], [1, NB]]),
                    bass.AP(zeros_sb[:].tensor, zeros_sb[:].offset, [[NB, 4], [1, NB]]),
                )
                nc.sync.dma_start(
                    bass.AP(gt[:].tensor, gt[:].offset + 1020 * GPITCH,
                            [[GPITCH, 4], [1, NB]]),
                    bass.AP(zeros_sb[:].tensor, zeros_sb[:].offset, [[NB, 4], [1, NB]]),
                )

            # ---------- phase C: per-head attention ----------
            outT_sb = [opool.tile([128, T], f32, tag=f"oT{ct}", name=f"oT{ct}") for ct in range(2)]
            d_sb = bpool.tile([NHEADS, T], f32, tag="dsb")
            abs4 = [bpool.tile([128, 64], f32, tag=f"abs4_{st}", name=f"abs4_{st}") for st in range(NT)]
            at_cat = bpool.tile([64, T], f32, tag="atcat")
            nc.gpsimd.memset(at_cat[:], 0.0)
            for st in range(NT):
                nc.gpsimd.memset(abs4[st][:], 0.0)
            with (
                tc.tile_pool(name="psS", bufs=2, space="PSUM") as psS,
                tc.tile_pool(name="psPV", bufs=1, space="PSUM") as psPV,
                tc.tile_pool(name="psT", bufs=2, space="PSUM") as psT,
            ):
                for h in range(NHEADS):
                    ct, r0 = h // 2, (h % 2) * 64
                    pv = psPV.tile([D + 1, T], f32, tag="pv")
                    et_tiles = []
                    for st in range(NT):
                        s0 = st * 128
                        sc = psS.tile([128, T], f32, tag="sc")
                        for nh in range(2):
                            nc.tensor.matmul(
                                sc[:, nh * 512:(nh + 1) * 512],
                                kT_sb[ct][r0:r0 + 64, s0:s0 + 128],
                                qsT_sb[ct][r0:r0 + 64, nh * 512:(nh + 1) * 512],
                                start=True, stop=True,
                            )
                        # band add: scatter S into dense window, add into psum
                        win = bpool.tile([128, 136], bf16, tag="win")
                        nc.gpsimd.local_scatter(
                            win[:], sbf[st][:, h * 10:h * 10 + 10],
                            sidx_sb[:], channels=128, num_elems=136, num_idxs=10,
                        )
                        lo = 4 if st == 0 else 0
                        hi = 132 if st == NT - 1 else 136
                        nc.vector.tensor_tensor(
                            sc[:, s0 - 4 + lo:s0 - 4 + hi],
                            sc[:, s0 - 4 + lo:s0 - 4 + hi],
                            win[:, lo:hi], op=AluAdd,
                        )
                        et = etpool.tile([128, T], f32, tag="et")
                        nc.scalar.activation(et[:], sc[:], Exp)
                        et_tiles.append(et)
                        # band window -> G (contiguous 544B runs, skewed layout)
                        gt = g_dram[h]
                        nc.sync.dma_start(
                            bass.AP(gt[:].tensor,
                                    gt[:].offset + s0 * GPITCH + lo,
                                    [[136, 128], [1, hi - lo]]),
                            et[:, s0 - 4 + lo:s0 - 4 + hi],
                        )
                        # compact band readback [128, 9] (36B runs)
                        nc.sync.dma_start(
                            abs4[st][:, h * 16:h * 16 + NB],
                            bass.AP(gt[:].tensor, gt[:].offset + s0 * GPITCH,
                                    [[GPITCH, 128], [1, NB]]),
                        )
                    # PV: out^T[d, t] (+ colsum in row 64) accumulated over s
                    for st in range(NT):
                        for nh in range(2):
                            nc.tensor.matmul(
                                pv[:, nh * 512:(nh + 1) * 512],
                                vaug_sb[st][:, h * 65:h * 65 + 65],
                                et_tiles[st][:, nh * 512:(nh + 1) * 512],
                                start=(st == 0), stop=False,
                            )
                    # rel-V: AbandT[9, t] via inverse skew, matmul into pv rows 0:64
                    for st in range(NT):
                        ps = psT.tile([64, 128], f32, tag="abt")
                        nc.tensor.transpose(
                            ps[:], abs4[st][:, :], ident[:]
                        )
                        nc.vector.tensor_copy(at_cat[:, st * 128:(st + 1) * 128], ps[:])
                    abt = bpool.tile([NB, T], f32, tag="abt9")
                    nc.gpsimd.memset(abt[:, 0:4], 0.0)
                    nc.gpsimd.memset(abt[:, T - 4:T], 0.0)
                    for m in range(NB):
                        if m < 4:
                            nc.sync.dma_start(
                                abt[8 - m:8 - m + 1, 0:1020 + m],
                                at_cat[h * 16 + m:h * 16 + m + 1, 4 - m:T],
                            )
                        else:
                            nc.sync.dma_start(
                                abt[8 - m:8 - m + 1, m - 4:T],
                                at_cat[h * 16 + m:h * 16 + m + 1, 0:T + 4 - m],
                            )
                    for nh in range(2):
                        nc.tensor.matmul(
                            pv[:, nh * 512:(nh + 1) * 512],
                            ev_sb[:],
                            abt[:, nh * 512:(nh + 1) * 512],
                            start=False, stop=True,
                        )
                    # evacuate
                    nc.vector.tensor_copy(outT_sb[ct][r0:r0 + 64, :], pv[0:D, :])
                    dtmp = bpool.tile([1, T], f32, tag="dtmp", name="dtmp")
                    nc.vector.tensor_copy(dtmp[:], pv[D:D + 1, :])
                    nc.sync.dma_start(d_sb[h:h + 1, :], dtmp[:])

            # ---------- phase D: recip + output projection ----------
            out_sb_pool = opool
            with (
                tc.tile_pool(name="psP", bufs=1, space="PSUM") as psP,
                tc.tile_pool(name="psD", bufs=2, space="PSUM") as psD,
            ):
                drt = bpool.tile([128, NT * NHEADS], f32, tag="drt")
                for st in range(NT):
                    ps = psD.tile([128, NHEADS], f32, tag="dtr")
                    nc.tensor.transpose(
                        ps[:], d_sb[:, st * 128:(st + 1) * 128],
                        ident[0:NHEADS, 0:NHEADS],
                    )
                    nc.vector.tensor_copy(
                        drt[:, st * NHEADS:(st + 1) * NHEADS], ps[:]
                    )
                nc.vector.reciprocal(drt[:], drt[:])
                for st in range(NT):
                    pps = []
                    for h in range(NHEADS):
                        ct, r0 = h // 2, (h % 2) * 64
                        pp = psP.tile([128, CIN], f32, tag=f"pj{h}")
                        nc.tensor.matmul(
                            pp[:],
                            outT_sb[ct][r0:r0 + 64, st * 128:(st + 1) * 128],
                            wo_sb[ct][r0:r0 + 64, :],
                            start=True, stop=True,
                        )
                        pps.append(pp)
                    acc = out_sb_pool.tile([128, CIN], f32, tag="acc")
                    nc.vector.tensor_scalar(
                        acc[:], pps[0][:],
                        drt[:, st * NHEADS:st * NHEADS + 1], None, op0=AluMult,
                    )
                    for h in range(1, NHEADS):
                        nc.vector.scalar_tensor_tensor(
                            acc[:], pps[h][:],
                            drt[:, st * NHEADS + h:st * NHEADS + h + 1],
                            acc[:], op0=AluMult, op1=AluAdd,
                        )
                    nc.sync.dma_start(out_p[st * 128:(st + 1) * 128, :], acc[:])

    nc.compile()
    return nc


def make_core_inputs(x, c, Wq, bq, Wk, bk, Wv, bv, Wo, bo, emb_rel_k, emb_rel_v,
                     core):
    b, hg = core // 2, core % 2
    sl = slice(hg * CH, (hg + 1) * CH)
    si = np.zeros((128, 10), np.int16)
    for p in range(128):
        for j in range(NB):
            si[p, j] = p + j
        si[p, 9] = -1
    return {
        "xT": np.ascontiguousarray(x[b].T).astype(np.float32),
        "cT": np.ascontiguousarray(c[b].T).astype(np.float32),
        "wq": np.ascontiguousarray(Wq[:, sl]).astype(np.float32),
        "wk": np.ascontiguousarray(Wk[:, sl]).astype(np.float32),
        "wv": np.ascontiguousarray(Wv[:, sl]).astype(np.float32),
        "wo": np.ascontiguousarray(Wo[sl, :]).astype(np.float32),
        "bq2": np.ascontiguousarray((bq[sl] * 0.125).reshape(2, 128).T).astype(np.float32),
        "bk2": np.ascontiguousarray(bk[sl].reshape(2, 128).T).astype(np.float32),
        "bv1": bv[sl].reshape(1, CH).astype(np.float32),
        "ekT": np.ascontiguousarray(emb_rel_k[0].T).astype(np.float32),
        "ev": np.ascontiguousarray(emb_rel_v[0]).astype(np.float32),
        "sidx": si,
    }


def kernel(**inputs):
    inputs = {k: np.asarray(v) for k, v in inputs.items()}
    nc = build_program()
    core_ids = list(range(8))
    in_maps = [make_core_inputs(core=i, **inputs) for i in core_ids]
    res = run_bass_kernel_spmd(nc, in_maps, core_ids).results
    B = inputs["x"].shape[0]
    out = np.zeros((B, T, CIN), np.float32)
    for b in range(B):
        out[b] = res[2 * b]["out_p"] + res[2 * b + 1]["out_p"] + inputs["bo"]
    return out

